# revision 1
# baseline (speedup 1.0000x reference)
"""LoRA multi-head attention on 8 TRN2 NeuronCores.

Sharding: data-parallel over batch (B=8 -> 1 batch element per core),
weights replicated, no collectives.

Host side (in kernel()): inputs are cast to bf16 and pre-transposed so
the device reads exactly the layouts the TensorEngine needs (the
contraction dim on partitions). Device DMA on this system is slow
(~30-100 GB/s aggregate), so shipping 12 MB of ready-to-use bf16
beats 40 MB of on-device cast/transpose traffic by a wide margin.

Device side per core, all bf16 with fp32 PSUM accumulation:
  qT = (WqT.T @ xT + BqT.T (AqT.T xT) / 16) / 8     [dout, n]
  kT likewise; v natural [n, dout] via (xT.T @ WvT), stored per-head
  with a ones column ([v_h | 1]) so PV also yields softmax denoms.
  Per head: S^T = kT_h.T qT_h -> exp (no max-sub; |s|=O(4)) -> PV;
  normalize via ones-outer-product broadcast + fast reciprocal.
  out = attnT.T @ WoT + lora + bo (bias via K=1 ones matmul).
"""

import sys

if "/opt/trn_rl_repo" not in sys.path:
    sys.path.insert(0, "/opt/trn_rl_repo")

import numpy as np
import ml_dtypes

BF16 = ml_dtypes.bfloat16

N = 1024  # tokens
D = 1024  # model dim
H = 16    # heads
HD = 64   # head dim
R = 16    # lora rank
P = 128   # partitions
F = 512   # psum free-dim tile
NCORES = 8
SCALING = 1.0 / 16.0  # lora alpha/rank
SCALE = HD ** -0.5

_CACHE = {}


def _build():
    import concourse.bacc as bacc
    import concourse.mybir as mybir
    import concourse.tile as tile

    f32 = mybir.dt.float32
    bf16 = mybir.dt.bfloat16
    Exp = mybir.ActivationFunctionType.Exp

    nc = bacc.Bacc("TRN2", target_bir_lowering=False, debug=False)

    # all big params arrive pre-transposed, bf16, from the host
    xT_e = nc.declare_dram_parameter("xT", [D, N], bf16, isOutput=False)
    wT_e = {
        nm: nc.declare_dram_parameter(nm, [D, D], bf16, isOutput=False)
        for nm in ("WqT", "WkT", "WvT", "WoT")
    }
    a3_e = nc.declare_dram_parameter("A3T", [D, 96], bf16, isOutput=False)
    b3_e = nc.declare_dram_parameter("B3T", [96, D], bf16, isOutput=False)
    aT_e = {
        nm: nc.declare_dram_parameter(nm, [D, R], bf16, isOutput=False)
        for nm in ("AoT",)
    }
    bT_e = {
        nm: nc.declare_dram_parameter(nm, [R + 1, D], bf16, isOutput=False)
        for nm in ("BoT",)
    }
    out_e = nc.declare_dram_parameter("out", [N, D], bf16, isOutput=True)

    with tile.TileContext(nc) as tc:
        with (
            tc.tile_pool(name="wpool", bufs=1) as wpool,
            tc.tile_pool(name="stage", bufs=2) as stage,
            tc.tile_pool(name="ps", bufs=1, space="PSUM") as ps,
        ):
            qs = [nc.sync, nc.scalar, nc.gpsimd]

            # ---- load pre-transposed tensors straight into SBUF ----
            T = {}
            aT = {}
            bT = {}
            qi = 0

            def load_big(nm, ext):
                nonlocal qi
                T[nm] = []
                for t in range(8):
                    tt = wpool.tile([P, D], bf16, tag=f"T_{nm}_{t}",
                                    name=f"T_{nm}_{t}")
                    qs[qi % 3].dma_start(out=tt[:],
                                         in_=ext[t * P:(t + 1) * P, :])
                    qi += 1
                    T[nm].append(tt)

            def load_a(nm):
                nonlocal qi
                key = nm[:2]
                aT[key] = []
                for t in range(8):
                    tt = wpool.tile([P, R], bf16, tag=f"aT_{nm}_{t}",
                                    name=f"aT_{nm}_{t}")
                    qs[qi % 3].dma_start(out=tt[:],
                                         in_=aT_e[nm][t * P:(t + 1) * P, :])
                    qi += 1
                    aT[key].append(tt)

            def load_b(nm):
                nonlocal qi
                tt = wpool.tile([R + 1, D], bf16, tag=f"bT_{nm}")
                qs[qi % 3].dma_start(out=tt[:], in_=bT_e[nm][:, :])
                qi += 1
                bT[nm[:2]] = tt

            load_big("x", xT_e)
            a3 = []
            for t in range(8):
                tt = wpool.tile([P, 96], bf16, tag=f"a3_{t}",
                                name=f"a3_{t}")
                qs[qi % 3].dma_start(out=tt[:],
                                     in_=a3_e[t * P:(t + 1) * P, :])
                qi += 1
                a3.append(tt)
            b3 = wpool.tile([96, D], bf16, tag="b3")
            qs[qi % 3].dma_start(out=b3[:], in_=b3_e[:, :])
            qi += 1
            load_big("Wv", wT_e["WvT"])
            load_big("Wq", wT_e["WqT"])
            load_big("Wk", wT_e["WkT"])
            load_big("Wo", wT_e["WoT"])
            load_a("AoT")
            load_b("BoT")
            onesf = wpool.tile([P, HD], f32, tag="onesf")
            nc.vector.memset(onesf[:], 1.0)

            # ---- PE warm-up, gated on the last x tile so it runs right
            # before dense compute (brings HAM to K=8/8) ----
            wps = ps.tile([P, F], f32, tag="tpsum", bufs=1)
            for _ in range(36):
                nc.tensor.matmul(wps[:], T["x"][7][:, 0:P],
                                 T["x"][7][:, 0:F], start=True, stop=True)

            # ---- lora intermediates, q/k/v packed at 32-aligned rows
            # (host ships A3T/B3T with Aq@0, Ak@32, Av@64) ----
            tsb3 = []
            for nh in range(2):
                ns = slice(nh * F, (nh + 1) * F)
                pt = ps.tile([96, F], f32, tag="tpsum", bufs=1)
                for kt in range(8):
                    nc.tensor.matmul(pt[:], a3[kt][:], T["x"][kt][:, ns],
                                     start=(kt == 0), stop=(kt == 7))
                t_s = stage.tile([96, F], bf16, tag="tsb", bufs=2,
                                 name=f"tsb3_{nh}")
                nc.vector.tensor_scalar_mul(t_s[:], pt[:], SCALING)
                tsb3.append(t_s)

            # ---- v natural, per-head layout [v_h | 1], with the dt=0
            # projection woven in so attention starts immediately after ----
            qks = {}

            def proj_gen(dt):
                qk = {}
                for nm, wnm, bnm, scl in (("q", "Wq", "Bq", SCALE),
                                          ("k", "Wk", "Bk", None)):
                    dst = wpool.tile([P, D], bf16, tag=f"{nm}T",
                                     bufs=3, name=f"{nm}T_{dt}")
                    qk[nm] = dst
                    for nh in range(2):
                        ns = slice(nh * F, (nh + 1) * F)
                        pq = ps.tile([P, F], f32, tag="projpsum", bufs=1)
                        for kt in range(8):
                            nc.tensor.matmul(
                                pq[:], T[wnm][kt][:, dt * P:(dt + 1) * P],
                                T["x"][kt][:, ns],
                                start=(kt == 0), stop=False)
                            yield
                        ro3 = 0 if nm == "q" else 32
                        nc.tensor.matmul(pq[:],
                                         b3[ro3:ro3 + R,
                                            dt * P:(dt + 1) * P],
                                         tsb3[nh][ro3:ro3 + R, :],
                                         start=False, stop=True)
                        yield
                        if scl is None:
                            nc.vector.tensor_copy(dst[:, ns], pq[:])
                        else:
                            nc.vector.tensor_scalar_mul(dst[:, ns],
                                                        pq[:], scl)
                        yield
                qks[dt] = qk

            VW = H * (HD + 1)  # 1040
            v_sb = [wpool.tile([P, VW], bf16, tag=f"v_{t}",
                               name=f"v_{t}") for t in range(8)]
            g0 = proj_gen(0)
            for nt in range(8):
                vr = v_sb[nt][:].rearrange("p (h c) -> p h c", c=HD + 1)
                for dh in range(2):
                    ds = slice(dh * F, (dh + 1) * F)
                    pv = ps.tile([P, F], f32, tag="spair", bufs=2)
                    for kt in range(8):
                        nc.tensor.matmul(
                            pv[:], T["x"][kt][:, nt * P:(nt + 1) * P],
                            T["Wv"][kt][:, ds], start=(kt == 0), stop=False)
                    nc.tensor.matmul(
                        pv[:],
                        tsb3[nt // 4][64:80, (nt % 4) * P:(nt % 4 + 1) * P],
                        b3[64:80, ds], start=False, stop=True)
                    pvr = pv[:].rearrange("p (h c) -> p h c", c=HD)
                    nc.vector.tensor_copy(vr[:, dh * 8:(dh + 1) * 8, 0:HD],
                                          pvr[:])
                    for _ in range(3):
                        next(g0, None)
                nc.vector.memset(vr[:, :, HD:HD + 1], 1.0)
            for _ in g0:
                pass

            # ---- per dout-tile: qT, kT, then its 2 heads' attention.
            # The NEXT tile's projection matmuls are woven into the
            # attention inner loop (generator) so the PE stays dense
            # while ACT runs the exps -- keeps HAM at K=8/8. ----
            attnT = [wpool.tile([P, D], bf16, tag=f"attnT_{t}",
                                name=f"attnT_{t}") for t in range(8)]
            for dt in range(8):
                g = proj_gen(dt + 1) if dt < 7 else iter(())
                h0 = 2 * dt
                qt = qks[dt]["q"]
                ktt = qks[dt]["k"]
                for nh in range(2):
                    ns = slice(nh * F, (nh + 1) * F)
                    po = {}
                    for h in (h0, h0 + 1):
                        po[h] = ps.tile([HD + 1, F], f32, tag="pvpsum",
                                        bufs=2, name=f"po_{h}_{nh}")
                    for mt in range(8):
                        spair = ps.tile([P, 2 * F], f32, tag="spair",
                                        bufs=2)
                        for hi, h in enumerate((h0, h0 + 1)):
                            ro = (h % 2) * HD
                            m0 = mt * P
                            nc.tensor.matmul(
                                spair[:, hi * F:(hi + 1) * F],
                                ktt[ro:ro + HD, m0:m0 + P],
                                qt[ro:ro + HD, ns], start=True, stop=True)
                        pte = stage.tile([P, 2 * F], bf16, tag="pt", bufs=3)
                        nc.scalar.activation(pte[:], spair[:], Exp)
                        for hi, h in enumerate((h0, h0 + 1)):
                            nc.tensor.matmul(
                                po[h][:],
                                v_sb[mt][:, h * (HD + 1):(h + 1) * (HD + 1)],
                                pte[:, hi * F:(hi + 1) * F],
                                start=(mt == 0), stop=(mt == 7))
                        for _ in range(3):
                            next(g, None)
                    for h in (h0, h0 + 1):
                        ro = (h % 2) * HD
                        oah = stage.tile([HD + 1, F], f32, tag="oah", bufs=3)
                        nc.vector.tensor_copy(oah[:], po[h][:])
                        pb = ps.tile([HD, F], f32, tag="tpsum", bufs=1)
                        nc.tensor.matmul(pb[:], onesf[HD:HD + 1, :],
                                         oah[HD:HD + 1, :],
                                         start=True, stop=True)
                        pbs = stage.tile([HD, F], f32, tag="pbs", bufs=3)
                        nc.vector.reciprocal_approx_fast(pbs[:], pb[:])
                        ast = stage.tile([HD, F], bf16, tag="ast", bufs=3)
                        nc.vector.tensor_mul(ast[:], oah[0:HD, :], pbs[:])
                        nc.sync.dma_start(out=attnT[dt][ro:ro + HD, ns],
                                          in_=ast[:])
                        for _ in range(2):
                            next(g, None)
                for _ in g:
                    pass

            # ---- output projection ----
            to = wpool.tile([R + 1, D], bf16, tag="toT")
            nc.vector.memset(to[:], 1.0)
            for nh in range(2):
                ns = slice(nh * F, (nh + 1) * F)
                pt = ps.tile([R, F], f32, tag="tpsum", bufs=1)
                for kt in range(8):
                    nc.tensor.matmul(pt[:], aT["Ao"][kt][:],
                                     attnT[kt][:, ns],
                                     start=(kt == 0), stop=(kt == 7))
                nc.vector.tensor_scalar_mul(to[0:R, ns], pt[:], SCALING)
            for nt in range(8):
                for dh in range(2):
                    ds = slice(dh * F, (dh + 1) * F)
                    pf = ps.tile([P, F], f32, tag="spair", bufs=2)
                    for kt in range(8):
                        nc.tensor.matmul(pf[:],
                                         attnT[kt][:, nt * P:(nt + 1) * P],
                                         T["Wo"][kt][:, ds],
                                         start=(kt == 0), stop=False)
                    nc.tensor.matmul(pf[:],
                                     to[0:R + 1, nt * P:(nt + 1) * P],
                                     bT["Bo"][0:R + 1, ds],
                                     start=False, stop=True)
                    osb = stage.tile([P, F], bf16, tag="osb")
                    nc.vector.tensor_copy(osb[:], pf[:])
                    nc.sync.dma_start(out=out_e[nt * P:(nt + 1) * P, ds],
                                      in_=osb[:])
    nc.compile()
    return nc


def _get_nc():
    if "nc" not in _CACHE:
        _CACHE["nc"] = _build()
    return _CACHE["nc"]


def _prep_shared(inputs):
    def tb(a):  # transpose + bf16, contiguous
        return np.ascontiguousarray(np.asarray(a, np.float32).T.astype(BF16))

    shared = {}
    for nm in ("Wq", "Wk", "Wv", "Wo", "Ao"):
        shared[nm + "T"] = tb(inputs[nm])
    boa = np.zeros((R + 1, D), np.float32)
    boa[0:R] = np.asarray(inputs["Bo"], np.float32).T
    boa[R] = np.asarray(inputs["bo"], np.float32)
    shared["BoT"] = np.ascontiguousarray(boa.astype(BF16))
    a3 = np.zeros((D, 96), np.float32)
    b3 = np.zeros((96, D), np.float32)
    for j, nm in enumerate(("q", "k", "v")):
        a3[:, 32 * j:32 * j + R] = np.asarray(inputs["A" + nm], np.float32).T
        b3[32 * j:32 * j + R, :] = np.asarray(inputs["B" + nm], np.float32).T
    shared["A3T"] = np.ascontiguousarray(a3.astype(BF16))
    shared["B3T"] = np.ascontiguousarray(b3.astype(BF16))
    return shared


def kernel(**inputs):
    from concourse import bass_utils

    nc = _get_nc()
    shared = _prep_shared(inputs)
    x = np.asarray(inputs["x"], np.float32)
    in_maps = []
    for i in range(NCORES):
        m = dict(shared)
        m["xT"] = np.ascontiguousarray(x[i].T.astype(BF16))
        in_maps.append(m)
    res = bass_utils.run_bass_kernel_spmd(nc, in_maps,
                                          core_ids=list(range(NCORES)))
    return np.stack([np.asarray(res.results[i]["out"]).astype(np.float32)
                     for i in range(NCORES)], axis=0)



# revision 10
# speedup vs baseline: 1.0007x; 1.0007x over previous
"""LoRA multi-head attention on 8 TRN2 NeuronCores.

Sharding: data-parallel over batch (B=8 -> 1 batch element per core),
weights replicated, no collectives.

Host side: LoRA is folded into the dense weights exactly
(W' = W + (alpha/r) * B @ A), the attention scale 1/8 is folded into
Wq (power of two => lossless), and bo is pre-replicated across
partitions. The device then runs a plain dense MHA in bf16 with fp32
PSUM accumulation.

Device pipeline per core:
  qT/kT per dout-tile dt: (WT.T @ xT); v natural per token-tile with a
  ones column ([v_h | 1]) so PV also yields softmax denominators.
  Attention per dt (2 heads), per query-half nh, per key-tile mt:
    S^T pair via 64-row PE tiling (both heads concurrent) -> exp on
    ACT (psum -> bf16 sbuf) -> PV (M=65).  The next dout-tile's q/k
    projection matmuls are woven between S and PV so the PE never
    waits on the ACT exp latency.
  Normalization: reciprocal of the denom row (bf16) broadcast via a
  K=1 bf16 matmul; DVE multiply writes attnT directly (head 0) or via
  a staging tile + SBUF DMA (head 1).
  Output projection: kt=0..6 partial sums woven into the dt=7
  attention loop (+bo), tail adds only the kt=7 term.
"""

import sys

if "/opt/trn_rl_repo" not in sys.path:
    sys.path.insert(0, "/opt/trn_rl_repo")

import numpy as np
import ml_dtypes

BF16 = ml_dtypes.bfloat16

N = 1024  # tokens
D = 1024  # model dim
H = 16    # heads
HD = 64   # head dim
R = 16    # lora rank
P = 128   # partitions
F = 512   # psum free-dim tile
NCORES = 8
SCALING = 1.0 / 16.0  # lora alpha/rank
SCALE = HD ** -0.5

_CACHE = {}


def _build():
    import concourse.bacc as bacc
    import concourse.mybir as mybir
    import concourse.tile as tile

    f32 = mybir.dt.float32
    bf16 = mybir.dt.bfloat16
    Exp = mybir.ActivationFunctionType.Exp

    nc = bacc.Bacc("TRN2", target_bir_lowering=False, debug=False)

    xT_e = nc.declare_dram_parameter("xT", [D, N], bf16, isOutput=False)
    wT_e = {
        nm: nc.declare_dram_parameter(nm, [D, D], bf16, isOutput=False)
        for nm in ("WqT", "WkT", "WvT", "WoT")
    }
    bo_e = nc.declare_dram_parameter("boR", [P, D], bf16, isOutput=False)
    out_e = nc.declare_dram_parameter("out", [N, D], bf16, isOutput=True)

    with tile.TileContext(nc) as tc:
        with (
            tc.tile_pool(name="wpool", bufs=1) as wpool,
            tc.tile_pool(name="stage", bufs=2) as stage,
            tc.tile_pool(name="ps", bufs=1, space="PSUM") as ps,
        ):
            # ---- DMA loads: x first (5 queues), then weights ----
            dq = [nc.sync, nc.scalar, nc.gpsimd]
            qi = 0
            T = {}

            def load_big(nm, ext, nq=3):
                nonlocal qi
                T[nm] = []
                for t in range(8):
                    tt = wpool.tile([P, D], bf16, tag=f"T_{nm}_{t}",
                                    name=f"T_{nm}_{t}")
                    dq[qi % nq].dma_start(out=tt[:],
                                          in_=ext[t * P:(t + 1) * P, :])
                    qi += 1
                    T[nm].append(tt)

            load_big("x", xT_e, nq=3)
            load_big("Wq", wT_e["WqT"])
            load_big("Wk", wT_e["WkT"])
            load_big("Wv", wT_e["WvT"])
            load_big("Wo", wT_e["WoT"])
            bo_sb = wpool.tile([P, D], bf16, tag="bo")
            nc.sync.dma_start(out=bo_sb[:], in_=bo_e[:, :])

            ones1 = wpool.tile([1, HD], bf16, tag="ones1")
            nc.vector.memset(ones1[:], 1.0)
            warm = wpool.tile([P, F], bf16, tag="warm")
            nc.vector.memset(warm[:], 0.0)

            # ---- ACT exp-table preload + PE warm-up (no DMA deps) ----
            wexp = stage.tile([P, F], bf16, tag="wexp", bufs=1)
            wps = ps.tile([P, F], f32, tag="wk", bufs=2)
            nc.tensor.matmul(wps[:], warm[:, 0:P], warm[:], start=True,
                             stop=True)
            nc.scalar.activation(wexp[:], wps[:], Exp)
            for _ in range(23):
                nc.tensor.matmul(wps[:], warm[:, 0:P], warm[:],
                                 start=True, stop=True)

            # ---- q/k projection generator for dout tile dt ----
            qks = {}

            def proj_gen(dt):
                qk = {}
                for nm, wnm in (("q", "Wq"), ("k", "Wk")):
                    dst = wpool.tile([P, D], bf16, tag=f"{nm}T",
                                     bufs=3, name=f"{nm}T_{dt}")
                    qk[nm] = dst
                    for nh in range(2):
                        ns = slice(nh * F, (nh + 1) * F)
                        pq = ps.tile([P, F], f32, tag="wk", bufs=2)
                        for kt in range(8):
                            nc.tensor.matmul(
                                pq[:], T[wnm][kt][:, dt * P:(dt + 1) * P],
                                T["x"][kt][:, ns],
                                start=(kt == 0), stop=(kt == 7))
                            yield
                        nc.vector.tensor_copy(dst[:, ns], pq[:])
                        yield
                qks[dt] = qk

            # ---- output-projection partials (kt=0..6) for dt=7 weave ----
            attnT = [wpool.tile([P, D], bf16, tag=f"attnT_{t}",
                                name=f"attnT_{t}") for t in range(8)]
            partials = [wpool.tile([P, F], bf16, tag=f"part_{t}",
                                   name=f"part_{t}") for t in range(16)]

            def out_gen():
                for nt in range(8):
                    for dh in range(2):
                        ds = slice(dh * F, (dh + 1) * F)
                        pf = ps.tile([P, F], f32, tag="wk", bufs=2)
                        for kt in range(7):
                            nc.tensor.matmul(
                                pf[:], attnT[kt][:, nt * P:(nt + 1) * P],
                                T["Wo"][kt][:, ds],
                                start=(kt == 0), stop=(kt == 6))
                            yield
                        nc.vector.tensor_add(
                            partials[nt * 2 + dh][:], pf[:], bo_sb[:, ds])
                        yield

            # ---- S-pair + exp issue (attention front half) ----
            PTE_BUFS = 13
            pmap = {}

            def s_exp(dt, nh, mt):
                qt = qks[dt]["q"]
                ktt = qks[dt]["k"]
                ns = slice(nh * F, (nh + 1) * F)
                m0 = mt * P
                spair = ps.tile([P, 2 * F], f32, tag="spair", bufs=2)
                nc.tensor.matmul(spair[:, 0:F], ktt[0:HD, m0:m0 + P],
                                 qt[0:HD, ns], start=True, stop=True)
                nc.tensor.matmul(spair[:, F:2 * F], ktt[HD:P, m0:m0 + P],
                                 qt[HD:P, ns], start=True, stop=True)
                pte = stage.tile([P, 2 * F], bf16, tag="pte",
                                 bufs=PTE_BUFS)
                nc.scalar.activation(pte[:], spair[:], Exp)
                pmap[(dt, nh, mt)] = pte

            # ---- v projection with dt=0 S/exp pre-issue woven in ----
            VW = H * (HD + 1)  # 1040
            v_sb = [wpool.tile([P, VW], bf16, tag=f"v_{t}",
                               name=f"v_{t}") for t in range(8)]
            g0 = proj_gen(0)
            for _ in g0:
                pass

            sched0 = [(0, nh, mt) for nh in range(2) for mt in range(8)]
            s0 = 0
            for nt in range(8):
                vr = v_sb[nt][:].rearrange("p (h c) -> p h c", c=HD + 1)
                for dh in range(2):
                    ds = slice(dh * F, (dh + 1) * F)
                    pv = ps.tile([P, F], f32, tag="wk", bufs=2)
                    for kt in range(8):
                        nc.tensor.matmul(
                            pv[:], T["x"][kt][:, nt * P:(nt + 1) * P],
                            T["Wv"][kt][:, ds], start=(kt == 0),
                            stop=(kt == 7))
                    pvr = pv[:].rearrange("p (h c) -> p h c", c=HD)
                    nc.vector.tensor_copy(vr[:, dh * 8:(dh + 1) * 8, 0:HD],
                                          pvr[:])
                    if s0 < 12 and nt >= 1:
                        s_exp(*sched0[s0])
                        s0 += 1
                nc.vector.memset(vr[:, :, HD:HD + 1], 1.0)

            # ---- attention + weave ----
            def norm(dt, nh, po0, po1):
                ns = slice(nh * F, (nh + 1) * F)
                for hi, po in ((0, po0), (1, po1)):
                    dnr = stage.tile([HD + 1, F], f32, tag="dnr", bufs=3)
                    nc.vector.tensor_copy(dnr[HD:HD + 1, :],
                                          po[HD:HD + 1, :])
                    dn0 = stage.tile([1, F], f32, tag="dn0", bufs=3)
                    nc.sync.dma_start(out=dn0[:], in_=dnr[HD:HD + 1, :])
                    rec32 = stage.tile([1, F], f32, tag="rec32", bufs=3)
                    nc.vector.reciprocal_approx_fast(rec32[:], dn0[:])
                    pbs = stage.tile([HD, F], f32, tag="pbs", bufs=3)
                    nc.gpsimd.partition_broadcast(pbs[:], rec32[0:1, :])
                    if hi == 0:
                        nc.vector.tensor_mul(attnT[dt][0:HD, ns],
                                             po[0:HD, :], pbs[:])
                    else:
                        ast = stage.tile([HD, F], bf16, tag="ast", bufs=3)
                        nc.vector.tensor_mul(ast[:], po[0:HD, :],
                                             pbs[:])
                        nc.sync.dma_start(out=attnT[dt][HD:P, ns],
                                          in_=ast[:])

            og = None
            for dt in range(8):
                if dt < 7:
                    g = proj_gen(dt + 1)
                    nw = 2
                else:
                    og = out_gen()
                    g = og
                    nw = 8
                h0 = 2 * dt
                for nh in range(2):
                    ns = slice(nh * F, (nh + 1) * F)
                    po0 = ps.tile([HD + 1, F], f32, tag="po", bufs=2,
                                  name=f"po0_{dt}_{nh}")
                    po1 = ps.tile([HD + 1, F], f32, tag="po", bufs=2,
                                  name=f"po1_{dt}_{nh}")
                    for mt in range(8):
                        if not (dt == 0 and (nh * 8 + mt) < 12):
                            s_exp(dt, nh, mt)
                        for _ in range(nw):
                            next(g, None)
                        pte = pmap.pop((dt, nh, mt))
                        nc.tensor.matmul(
                            po0[:], v_sb[mt][:, h0 * (HD + 1):
                                             (h0 + 1) * (HD + 1)],
                            pte[:, 0:F], start=(mt == 0), stop=(mt == 7))
                        nc.tensor.matmul(
                            po1[:], v_sb[mt][:, (h0 + 1) * (HD + 1):
                                             (h0 + 2) * (HD + 1)],
                            pte[:, F:2 * F], start=(mt == 0), stop=(mt == 7))
                    norm(dt, nh, po0, po1)
                if dt < 7:
                    for _ in g:
                        pass

            # ---- output projection tail: kt=7 term + partial ----
            for _ in og:
                pass
            for nt in range(8):
                for dh in range(2):
                    ds = slice(dh * F, (dh + 1) * F)
                    pf = ps.tile([P, F], f32, tag="wk", bufs=2)
                    nc.tensor.matmul(pf[:],
                                     attnT[7][:, nt * P:(nt + 1) * P],
                                     T["Wo"][7][:, ds],
                                     start=True, stop=True)
                    osb = stage.tile([P, F], bf16, tag="osb", bufs=3)
                    nc.vector.tensor_add(osb[:], pf[:],
                                         partials[nt * 2 + dh][:])
                    nc.sync.dma_start(out=out_e[nt * P:(nt + 1) * P, ds],
                                      in_=osb[:])
    nc.compile()
    return nc


def _get_nc():
    if "nc" not in _CACHE:
        _CACHE["nc"] = _build()
    return _CACHE["nc"]


def _prep_shared(inputs):
    f = lambda a: np.asarray(a, np.float32)
    W = {}
    W["q"] = (f(inputs["Wq"]) + SCALING * (f(inputs["Bq"]) @ f(inputs["Aq"]))) * SCALE
    W["k"] = f(inputs["Wk"]) + SCALING * (f(inputs["Bk"]) @ f(inputs["Ak"]))
    W["v"] = f(inputs["Wv"]) + SCALING * (f(inputs["Bv"]) @ f(inputs["Av"]))
    W["o"] = f(inputs["Wo"]) + SCALING * (f(inputs["Bo"]) @ f(inputs["Ao"]))
    shared = {}
    for k, nm in (("q", "WqT"), ("k", "WkT"), ("v", "WvT"), ("o", "WoT")):
        shared[nm] = np.ascontiguousarray(W[k].T.astype(BF16))
    bo = f(inputs["bo"]).reshape(1, D)
    shared["boR"] = np.ascontiguousarray(
        np.broadcast_to(bo, (P, D)).astype(BF16))
    return shared


def kernel(**inputs):
    from concourse import bass_utils

    nc = _get_nc()
    shared = _prep_shared(inputs)
    x = np.asarray(inputs["x"], np.float32)
    in_maps = []
    for i in range(NCORES):
        m = dict(shared)
        m["xT"] = np.ascontiguousarray(x[i].T.astype(BF16))
        in_maps.append(m)
    res = bass_utils.run_bass_kernel_spmd(nc, in_maps,
                                          core_ids=list(range(NCORES)))
    return np.stack([np.asarray(res.results[i]["out"]).astype(np.float32)
                     for i in range(NCORES)], axis=0)


# revision 13
# speedup vs baseline: 1.1691x; 1.1683x over previous
"""LoRA multi-head attention on 8 TRN2 NeuronCores.

Sharding: data-parallel over batch (B=8 -> 1 batch element per core),
weights replicated, no collectives.

Host side: LoRA is folded into the dense weights exactly
(W' = W + (alpha/r) * B @ A), the attention scale 1/8 is folded into
Wq (power of two => lossless), and bo is pre-replicated across
partitions. The device then runs a plain dense MHA in bf16 with fp32
PSUM accumulation.

Device pipeline per core:
  qT/kT per dout-tile dt: (WT.T @ xT); v natural per token-tile with a
  ones column ([v_h | 1]) so PV also yields softmax denominators.
  Attention per dt (2 heads), per query-half nh, per key-tile mt:
    S^T pair via 64-row PE tiling (both heads concurrent) -> exp on
    ACT (psum -> bf16 sbuf) -> PV (M=65).  The next dout-tile's q/k
    projection matmuls are woven between S and PV so the PE never
    waits on the ACT exp latency.
  Normalization: reciprocal of the denom row (bf16) broadcast via a
  K=1 bf16 matmul; DVE multiply writes attnT directly (head 0) or via
  a staging tile + SBUF DMA (head 1).
  Output projection: kt=0..6 partial sums woven into the dt=7
  attention loop (+bo), tail adds only the kt=7 term.
"""

import sys

if "/opt/trn_rl_repo" not in sys.path:
    sys.path.insert(0, "/opt/trn_rl_repo")

import numpy as np
import ml_dtypes

BF16 = ml_dtypes.bfloat16

N = 1024  # tokens
D = 1024  # model dim
H = 16    # heads
HD = 64   # head dim
R = 16    # lora rank
P = 128   # partitions
F = 512   # psum free-dim tile
NCORES = 8
SCALING = 1.0 / 16.0  # lora alpha/rank
SCALE = HD ** -0.5

_CACHE = {}


def _build():
    import concourse.bacc as bacc
    import concourse.mybir as mybir
    import concourse.tile as tile

    f32 = mybir.dt.float32
    bf16 = mybir.dt.bfloat16
    Exp = mybir.ActivationFunctionType.Exp

    nc = bacc.Bacc("TRN2", target_bir_lowering=False, debug=False)

    xT_e = nc.declare_dram_parameter("xT", [D, N], bf16, isOutput=False)
    wT_e = {
        nm: nc.declare_dram_parameter(nm, [D, D], bf16, isOutput=False)
        for nm in ("WqT", "WkT", "WvT", "WoT")
    }
    bo_e = nc.declare_dram_parameter("boR", [P, D], bf16, isOutput=False)
    out_e = nc.declare_dram_parameter("out", [N, D], bf16, isOutput=True)

    with tile.TileContext(nc) as tc:
        with (
            tc.tile_pool(name="wpool", bufs=1) as wpool,
            tc.tile_pool(name="stage", bufs=2) as stage,
            tc.tile_pool(name="ps", bufs=1, space="PSUM") as ps,
        ):
            # ---- DMA loads: x first (5 queues), then weights ----
            dq = [nc.sync, nc.scalar, nc.gpsimd]
            qi = 0
            T = {}

            def load_big(nm, ext, q=None):
                nonlocal qi
                T[nm] = []
                for t in range(8):
                    tt = wpool.tile([P, D], bf16, tag=f"T_{nm}_{t}",
                                    name=f"T_{nm}_{t}")
                    eng = dq[qi % 3] if q is None else q
                    eng.dma_start(out=tt[:], in_=ext[t * P:(t + 1) * P, :])
                    qi += 1
                    T[nm].append(tt)

            # x / Wq / Wk land concurrently so dt=0 attention starts early
            load_big("x", xT_e, q=nc.sync)
            load_big("Wq", wT_e["WqT"], q=nc.scalar)
            load_big("Wk", wT_e["WkT"], q=nc.gpsimd)
            load_big("Wv", wT_e["WvT"])
            load_big("Wo", wT_e["WoT"])
            bo_sb = wpool.tile([P, D], bf16, tag="bo")
            nc.sync.dma_start(out=bo_sb[:], in_=bo_e[:, :])

            ones1 = wpool.tile([1, HD], bf16, tag="ones1")
            nc.vector.memset(ones1[:], 1.0)
            warm = wpool.tile([P, F], bf16, tag="warm")
            nc.vector.memset(warm[:], 0.0)

            # ---- ACT exp-table preload + PE warm-up (no DMA deps) ----
            wexp = stage.tile([P, F], bf16, tag="wexp", bufs=1)
            wps = ps.tile([P, F], f32, tag="wk", bufs=2)
            nc.tensor.matmul(wps[:], warm[:, 0:P], warm[:], start=True,
                             stop=True)
            nc.scalar.activation(wexp[:], wps[:], Exp)
            for _ in range(23):
                nc.tensor.matmul(wps[:], warm[:, 0:P], warm[:],
                                 start=True, stop=True)

            # ---- q/k projection generator for dout tile dt ----
            qks = {}

            def proj_gen(dt):
                qk = {}
                for nm, wnm in (("q", "Wq"), ("k", "Wk")):
                    dst = wpool.tile([P, D], bf16, tag=f"{nm}T",
                                     bufs=3, name=f"{nm}T_{dt}")
                    qk[nm] = dst
                    for nh in range(2):
                        ns = slice(nh * F, (nh + 1) * F)
                        pq = ps.tile([P, F], f32, tag="wk", bufs=2)
                        for kt in range(8):
                            nc.tensor.matmul(
                                pq[:], T[wnm][kt][:, dt * P:(dt + 1) * P],
                                T["x"][kt][:, ns],
                                start=(kt == 0), stop=(kt == 7))
                            yield
                        nc.vector.tensor_copy(dst[:, ns], pq[:])
                        yield
                qks[dt] = qk

            # ---- output-projection partials (kt=0..6) for dt=7 weave ----
            attnT = [wpool.tile([P, D], bf16, tag=f"attnT_{t}",
                                name=f"attnT_{t}") for t in range(8)]
            partials = [wpool.tile([P, F], bf16, tag=f"part_{t}",
                                   name=f"part_{t}") for t in range(16)]

            def out_gen():
                for nt in range(8):
                    for dh in range(2):
                        ds = slice(dh * F, (dh + 1) * F)
                        pf = ps.tile([P, F], f32, tag="wk", bufs=2)
                        for kt in range(7):
                            nc.tensor.matmul(
                                pf[:], attnT[kt][:, nt * P:(nt + 1) * P],
                                T["Wo"][kt][:, ds],
                                start=(kt == 0), stop=(kt == 6))
                            yield
                        nc.vector.tensor_add(
                            partials[nt * 2 + dh][:], pf[:], bo_sb[:, ds])
                        yield

            # ---- S-pair + exp issue (attention front half) ----
            PTE_BUFS = 13
            pmap = {}

            def s_exp(dt, nh, mt):
                qt = qks[dt]["q"]
                ktt = qks[dt]["k"]
                ns = slice(nh * F, (nh + 1) * F)
                m0 = mt * P
                spair = ps.tile([P, 2 * F], f32, tag="spair", bufs=2)
                nc.tensor.matmul(spair[:, 0:F], ktt[0:HD, m0:m0 + P],
                                 qt[0:HD, ns], start=True, stop=True)
                nc.tensor.matmul(spair[:, F:2 * F], ktt[HD:P, m0:m0 + P],
                                 qt[HD:P, ns], start=True, stop=True)
                pte = stage.tile([P, 2 * F], bf16, tag="pte",
                                 bufs=PTE_BUFS)
                nc.scalar.activation(pte[:], spair[:], Exp)
                pmap[(dt, nh, mt)] = pte

            # ---- v projection with dt=0 S/exp pre-issue woven in ----
            VW = H * (HD + 1)  # 1040
            v_sb = [wpool.tile([P, VW], bf16, tag=f"v_{t}",
                               name=f"v_{t}") for t in range(8)]
            g0 = proj_gen(0)
            for _ in g0:
                pass

            sched0 = [(0, nh, mt) for nh in range(2) for mt in range(8)]
            s0 = 0
            for nt in range(8):
                vr = v_sb[nt][:].rearrange("p (h c) -> p h c", c=HD + 1)
                for dh in range(2):
                    ds = slice(dh * F, (dh + 1) * F)
                    pv = ps.tile([P, F], f32, tag="wk", bufs=2)
                    for kt in range(8):
                        nc.tensor.matmul(
                            pv[:], T["x"][kt][:, nt * P:(nt + 1) * P],
                            T["Wv"][kt][:, ds], start=(kt == 0),
                            stop=(kt == 7))
                    pvr = pv[:].rearrange("p (h c) -> p h c", c=HD)
                    nc.vector.tensor_copy(vr[:, dh * 8:(dh + 1) * 8, 0:HD],
                                          pvr[:])
                    if s0 < 12 and nt >= 1:
                        s_exp(*sched0[s0])
                        s0 += 1
                nc.vector.memset(vr[:, :, HD:HD + 1], 1.0)

            # ---- attention + weave ----
            def norm(dt, nh, po0, po1):
                ns = slice(nh * F, (nh + 1) * F)
                for hi, po in ((0, po0), (1, po1)):
                    # one copy frees the po PSUM bank; rest runs from SBUF
                    oah = stage.tile([HD + 1, F], f32, tag="oah", bufs=4)
                    nc.vector.tensor_copy(oah[:], po[:])
                    dn0 = stage.tile([1, F], f32, tag="dn0", bufs=3)
                    nc.sync.dma_start(out=dn0[:], in_=oah[HD:HD + 1, :])
                    rec32 = stage.tile([1, F], f32, tag="rec32", bufs=3)
                    nc.vector.reciprocal_approx_fast(rec32[:], dn0[:])
                    pbs = stage.tile([HD, F], f32, tag="pbs", bufs=3)
                    nc.gpsimd.partition_broadcast(pbs[:], rec32[0:1, :])
                    if hi == 0:
                        nc.vector.tensor_mul(attnT[dt][0:HD, ns],
                                             oah[0:HD, :], pbs[:])
                    else:
                        ast = stage.tile([HD, F], bf16, tag="ast", bufs=3)
                        nc.vector.tensor_mul(ast[:], oah[0:HD, :],
                                             pbs[:])
                        nc.sync.dma_start(out=attnT[dt][HD:P, ns],
                                          in_=ast[:])

            og = None
            for dt in range(8):
                if dt < 7:
                    g = proj_gen(dt + 1)
                    nw = 2
                else:
                    og = out_gen()
                    g = og
                    nw = 8
                h0 = 2 * dt
                for nh in range(2):
                    ns = slice(nh * F, (nh + 1) * F)
                    po0 = ps.tile([HD + 1, F], f32, tag="po", bufs=2,
                                  name=f"po0_{dt}_{nh}")
                    po1 = ps.tile([HD + 1, F], f32, tag="po", bufs=2,
                                  name=f"po1_{dt}_{nh}")
                    for mt in range(8):
                        if not (dt == 0 and (nh * 8 + mt) < 12):
                            s_exp(dt, nh, mt)
                        for _ in range(nw):
                            next(g, None)
                        pte = pmap.pop((dt, nh, mt))
                        nc.tensor.matmul(
                            po0[:], v_sb[mt][:, h0 * (HD + 1):
                                             (h0 + 1) * (HD + 1)],
                            pte[:, 0:F], start=(mt == 0), stop=(mt == 7))
                        nc.tensor.matmul(
                            po1[:], v_sb[mt][:, (h0 + 1) * (HD + 1):
                                             (h0 + 2) * (HD + 1)],
                            pte[:, F:2 * F], start=(mt == 0), stop=(mt == 7))
                    norm(dt, nh, po0, po1)
                if dt < 7:
                    for _ in g:
                        pass

            # ---- output projection tail: kt=7 term + partial ----
            for _ in og:
                pass
            for nt in range(8):
                for dh in range(2):
                    ds = slice(dh * F, (dh + 1) * F)
                    pf = ps.tile([P, F], f32, tag="wk", bufs=2)
                    nc.tensor.matmul(pf[:],
                                     attnT[7][:, nt * P:(nt + 1) * P],
                                     T["Wo"][7][:, ds],
                                     start=True, stop=True)
                    osb = stage.tile([P, F], bf16, tag="osb", bufs=4)
                    nc.vector.tensor_add(osb[:], pf[:],
                                         partials[nt * 2 + dh][:])
                    dq[(nt * 2 + dh) % 3].dma_start(
                        out=out_e[nt * P:(nt + 1) * P, ds], in_=osb[:])
    nc.compile()
    return nc


def _get_nc():
    if "nc" not in _CACHE:
        _CACHE["nc"] = _build()
    return _CACHE["nc"]


def _prep_shared(inputs):
    f = lambda a: np.asarray(a, np.float32)
    W = {}
    W["q"] = (f(inputs["Wq"]) + SCALING * (f(inputs["Bq"]) @ f(inputs["Aq"]))) * SCALE
    W["k"] = f(inputs["Wk"]) + SCALING * (f(inputs["Bk"]) @ f(inputs["Ak"]))
    W["v"] = f(inputs["Wv"]) + SCALING * (f(inputs["Bv"]) @ f(inputs["Av"]))
    W["o"] = f(inputs["Wo"]) + SCALING * (f(inputs["Bo"]) @ f(inputs["Ao"]))
    shared = {}
    for k, nm in (("q", "WqT"), ("k", "WkT"), ("v", "WvT"), ("o", "WoT")):
        shared[nm] = np.ascontiguousarray(W[k].T.astype(BF16))
    bo = f(inputs["bo"]).reshape(1, D)
    shared["boR"] = np.ascontiguousarray(
        np.broadcast_to(bo, (P, D)).astype(BF16))
    return shared


def kernel(**inputs):
    from concourse import bass_utils

    nc = _get_nc()
    shared = _prep_shared(inputs)
    x = np.asarray(inputs["x"], np.float32)
    in_maps = []
    for i in range(NCORES):
        m = dict(shared)
        m["xT"] = np.ascontiguousarray(x[i].T.astype(BF16))
        in_maps.append(m)
    res = bass_utils.run_bass_kernel_spmd(nc, in_maps,
                                          core_ids=list(range(NCORES)))
    return np.stack([np.asarray(res.results[i]["out"]).astype(np.float32)
                     for i in range(NCORES)], axis=0)


# revision 16
# speedup vs baseline: 1.1716x; 1.0022x over previous
"""LoRA multi-head attention on 8 TRN2 NeuronCores.

Sharding: data-parallel over batch (B=8 -> 1 batch element per core),
weights replicated, no collectives.

Host side: LoRA is folded into the dense weights exactly
(W' = W + (alpha/r) * B @ A), the attention scale 1/8 is folded into
Wq (power of two => lossless), and bo is pre-replicated across
partitions. The device then runs a plain dense MHA in bf16 with fp32
PSUM accumulation.

Device pipeline per core:
  qT/kT per dout-tile dt: (WT.T @ xT); v natural per token-tile with a
  ones column ([v_h | 1]) so PV also yields softmax denominators.
  Attention per dt (2 heads), per query-half nh, per key-tile mt:
    S^T pair via 64-row PE tiling (both heads concurrent) -> exp on
    ACT (psum -> bf16 sbuf) -> PV (M=65).  The next dout-tile's q/k
    projection matmuls are woven between S and PV so the PE never
    waits on the ACT exp latency.
  Normalization: reciprocal of the denom row (bf16) broadcast via a
  K=1 bf16 matmul; DVE multiply writes attnT directly (head 0) or via
  a staging tile + SBUF DMA (head 1).
  Output projection: kt=0..6 partial sums woven into the dt=7
  attention loop (+bo), tail adds only the kt=7 term.
"""

import sys

if "/opt/trn_rl_repo" not in sys.path:
    sys.path.insert(0, "/opt/trn_rl_repo")

import numpy as np
import ml_dtypes

BF16 = ml_dtypes.bfloat16

N = 1024  # tokens
D = 1024  # model dim
H = 16    # heads
HD = 64   # head dim
R = 16    # lora rank
P = 128   # partitions
F = 512   # psum free-dim tile
NCORES = 8
SCALING = 1.0 / 16.0  # lora alpha/rank
SCALE = HD ** -0.5

_CACHE = {}


def _build():
    import concourse.bacc as bacc
    import concourse.mybir as mybir
    import concourse.tile as tile

    f32 = mybir.dt.float32
    bf16 = mybir.dt.bfloat16
    Exp = mybir.ActivationFunctionType.Exp

    nc = bacc.Bacc("TRN2", target_bir_lowering=False, debug=False)

    xT_e = nc.declare_dram_parameter("xT", [D, N], bf16, isOutput=False)
    wT_e = {
        nm: nc.declare_dram_parameter(nm, [D, D], bf16, isOutput=False)
        for nm in ("WqT", "WkT", "WvT", "WoT")
    }
    bo_e = nc.declare_dram_parameter("boR", [P, D], bf16, isOutput=False)
    out_e = nc.declare_dram_parameter("out", [N, D], bf16, isOutput=True)

    with tile.TileContext(nc) as tc:
        with (
            tc.tile_pool(name="wpool", bufs=1) as wpool,
            tc.tile_pool(name="stage", bufs=2) as stage,
            tc.tile_pool(name="ps", bufs=1, space="PSUM") as ps,
        ):
            # ---- DMA loads: x first (5 queues), then weights ----
            dq = [nc.sync, nc.scalar, nc.gpsimd]
            qi = 0
            T = {}

            def load_big(nm, ext, q=None):
                nonlocal qi
                T[nm] = []
                for t in range(8):
                    tt = wpool.tile([P, D], bf16, tag=f"T_{nm}_{t}",
                                    name=f"T_{nm}_{t}")
                    eng = dq[qi % 3] if q is None else q
                    eng.dma_start(out=tt[:], in_=ext[t * P:(t + 1) * P, :])
                    qi += 1
                    T[nm].append(tt)

            # x first; Wq/Wk column-sliced per dt so dt=0 projections can
            # start as soon as ~0.5MB has landed; Wv next (v-projection);
            # later dt columns and Wo trickle in behind compute.
            load_big("x", xT_e)

            def load_qk_cols(dts):
                nonlocal qi
                for dt in dts:
                    for nm in ("Wq", "Wk"):
                        if nm not in T:
                            T[nm] = [wpool.tile([P, D], bf16,
                                                tag=f"T_{nm}_{t}",
                                                name=f"T_{nm}_{t}")
                                     for t in range(8)]
                        for t in range(8):
                            cs = slice(dt * P, (dt + 1) * P)
                            dq[qi % 3].dma_start(
                                out=T[nm][t][:, cs],
                                in_=wT_e[nm + "T"][t * P:(t + 1) * P, cs])
                            qi += 1

            load_qk_cols([0])
            load_big("Wv", wT_e["WvT"])
            load_qk_cols([1, 2, 3, 4, 5, 6, 7])
            load_big("Wo", wT_e["WoT"])
            bo_sb = wpool.tile([P, D], bf16, tag="bo")
            nc.sync.dma_start(out=bo_sb[:], in_=bo_e[:, :])

            ones1 = wpool.tile([1, HD], bf16, tag="ones1")
            nc.vector.memset(ones1[:], 1.0)
            warm = wpool.tile([P, F], bf16, tag="warm")
            nc.vector.memset(warm[:], 0.0)

            # ---- ACT exp-table preload + PE warm-up (no DMA deps) ----
            wexp = stage.tile([P, F], bf16, tag="wexp", bufs=1)
            wps = ps.tile([P, F], f32, tag="wk", bufs=2)
            nc.tensor.matmul(wps[:], warm[:, 0:P], warm[:], start=True,
                             stop=True)
            nc.scalar.activation(wexp[:], wps[:], Exp)
            for _ in range(23):
                nc.tensor.matmul(wps[:], warm[:, 0:P], warm[:],
                                 start=True, stop=True)

            # ---- q/k projection generator for dout tile dt ----
            qks = {}

            def proj_gen(dt):
                qk = {}
                for nm, wnm in (("q", "Wq"), ("k", "Wk")):
                    dst = wpool.tile([P, D], bf16, tag=f"{nm}T",
                                     bufs=3, name=f"{nm}T_{dt}")
                    qk[nm] = dst
                    for nh in range(2):
                        ns = slice(nh * F, (nh + 1) * F)
                        pq = ps.tile([P, F], f32, tag="wk", bufs=2)
                        for kt in range(8):
                            nc.tensor.matmul(
                                pq[:], T[wnm][kt][:, dt * P:(dt + 1) * P],
                                T["x"][kt][:, ns],
                                start=(kt == 0), stop=(kt == 7))
                            yield
                        nc.vector.tensor_copy(dst[:, ns], pq[:])
                        yield
                qks[dt] = qk

            # ---- output-projection partials (kt=0..6) for dt=7 weave ----
            attnT = [wpool.tile([P, D], bf16, tag=f"attnT_{t}",
                                name=f"attnT_{t}") for t in range(8)]
            partials = [wpool.tile([P, F], bf16, tag=f"part_{t}",
                                   name=f"part_{t}") for t in range(16)]

            def out_gen():
                for nt in range(8):
                    for dh in range(2):
                        ds = slice(dh * F, (dh + 1) * F)
                        pf = ps.tile([P, F], f32, tag="wk", bufs=2)
                        for kt in range(7):
                            nc.tensor.matmul(
                                pf[:], attnT[kt][:, nt * P:(nt + 1) * P],
                                T["Wo"][kt][:, ds],
                                start=(kt == 0), stop=(kt == 6))
                            yield
                        nc.vector.tensor_add(
                            partials[nt * 2 + dh][:], pf[:], bo_sb[:, ds])
                        yield

            # ---- S-pair + exp issue (attention front half) ----
            PTE_BUFS = 13
            pmap = {}

            def s_exp(dt, nh, mt):
                qt = qks[dt]["q"]
                ktt = qks[dt]["k"]
                ns = slice(nh * F, (nh + 1) * F)
                m0 = mt * P
                spair = ps.tile([P, 2 * F], f32, tag="spair", bufs=2)
                nc.tensor.matmul(spair[:, 0:F], ktt[0:HD, m0:m0 + P],
                                 qt[0:HD, ns], start=True, stop=True)
                nc.tensor.matmul(spair[:, F:2 * F], ktt[HD:P, m0:m0 + P],
                                 qt[HD:P, ns], start=True, stop=True)
                pte = stage.tile([P, 2 * F], bf16, tag="pte",
                                 bufs=PTE_BUFS)
                nc.scalar.activation(pte[:], spair[:], Exp)
                pmap[(dt, nh, mt)] = pte

            # ---- v projection with dt=0 S/exp pre-issue woven in ----
            VW = H * (HD + 1)  # 1040
            v_sb = [wpool.tile([P, VW], bf16, tag=f"v_{t}",
                               name=f"v_{t}") for t in range(8)]
            g0 = proj_gen(0)
            for _ in g0:
                pass

            sched0 = [(0, nh, mt) for nh in range(2) for mt in range(8)]
            s0 = 0
            for nt in range(8):
                vr = v_sb[nt][:].rearrange("p (h c) -> p h c", c=HD + 1)
                for dh in range(2):
                    ds = slice(dh * F, (dh + 1) * F)
                    pv = ps.tile([P, F], f32, tag="wk", bufs=2)
                    for kt in range(8):
                        nc.tensor.matmul(
                            pv[:], T["x"][kt][:, nt * P:(nt + 1) * P],
                            T["Wv"][kt][:, ds], start=(kt == 0),
                            stop=(kt == 7))
                    pvr = pv[:].rearrange("p (h c) -> p h c", c=HD)
                    nc.vector.tensor_copy(vr[:, dh * 8:(dh + 1) * 8, 0:HD],
                                          pvr[:])
                    if s0 < 12 and nt >= 1:
                        s_exp(*sched0[s0])
                        s0 += 1
                nc.vector.memset(vr[:, :, HD:HD + 1], 1.0)

            # ---- attention + weave ----
            def norm(dt, nh, po0, po1):
                ns = slice(nh * F, (nh + 1) * F)
                for hi, po in ((0, po0), (1, po1)):
                    # one copy frees the po PSUM bank; rest runs from SBUF
                    oah = stage.tile([HD + 1, F], f32, tag="oah", bufs=4)
                    nc.vector.tensor_copy(oah[:], po[:])
                    dn0 = stage.tile([1, F], f32, tag="dn0", bufs=3)
                    nc.sync.dma_start(out=dn0[:], in_=oah[HD:HD + 1, :])
                    rec32 = stage.tile([1, F], f32, tag="rec32", bufs=3)
                    nc.vector.reciprocal_approx_fast(rec32[:], dn0[:])
                    pbs = stage.tile([HD, F], f32, tag="pbs", bufs=3)
                    nc.gpsimd.partition_broadcast(pbs[:], rec32[0:1, :])
                    if hi == 0:
                        nc.vector.tensor_mul(attnT[dt][0:HD, ns],
                                             oah[0:HD, :], pbs[:])
                    else:
                        ast = stage.tile([HD, F], bf16, tag="ast", bufs=3)
                        nc.vector.tensor_mul(ast[:], oah[0:HD, :],
                                             pbs[:])
                        nc.sync.dma_start(out=attnT[dt][HD:P, ns],
                                          in_=ast[:])

            # Flat software pipeline over all 128 (dt, nh, mt) steps:
            # S(j) issues one step ahead of PV(j-1) so ACT runs exp
            # back-to-back across block boundaries.
            steps = [(dt, nh, mt) for dt in range(8) for nh in range(2)
                     for mt in range(8)]
            gens = {dt: (proj_gen(dt + 1) if dt < 7 else out_gen())
                    for dt in range(8)}
            pos = {}
            for j in range(129):
                if j >= 1 and j < 128 and steps[j - 1][1:] == (1, 7):
                    # dt boundary: finish the next dt's q/k projection
                    # (python-level: binds qks[dt+1]) before issuing its S
                    for _ in gens[steps[j - 1][0]]:
                        pass
                if j < 128:
                    dt, nh, mt = steps[j]
                    if mt == 0:
                        pos[(dt, nh)] = (
                            ps.tile([HD + 1, F], f32, tag="po", bufs=2,
                                    name=f"po0_{dt}_{nh}"),
                            ps.tile([HD + 1, F], f32, tag="po", bufs=2,
                                    name=f"po1_{dt}_{nh}"))
                    if not (dt == 0 and (nh * 8 + mt) < 12):
                        s_exp(dt, nh, mt)
                if j >= 1:
                    dt, nh, mt = steps[j - 1]
                    g = gens[dt]
                    nw = 8 if dt == 7 else 2
                    for _ in range(nw):
                        next(g, None)
                    h0 = 2 * dt
                    po0, po1 = pos[(dt, nh)]
                    pte = pmap.pop((dt, nh, mt))
                    nc.tensor.matmul(
                        po0[:], v_sb[mt][:, h0 * (HD + 1):
                                         (h0 + 1) * (HD + 1)],
                        pte[:, 0:F], start=(mt == 0), stop=(mt == 7))
                    nc.tensor.matmul(
                        po1[:], v_sb[mt][:, (h0 + 1) * (HD + 1):
                                         (h0 + 2) * (HD + 1)],
                        pte[:, F:2 * F], start=(mt == 0), stop=(mt == 7))
                    if mt == 7:
                        norm(dt, nh, po0, po1)

            # ---- output projection tail: kt=7 term + partial ----
            for nt in range(8):
                for dh in range(2):
                    ds = slice(dh * F, (dh + 1) * F)
                    pf = ps.tile([P, F], f32, tag="wk", bufs=2)
                    nc.tensor.matmul(pf[:],
                                     attnT[7][:, nt * P:(nt + 1) * P],
                                     T["Wo"][7][:, ds],
                                     start=True, stop=True)
                    osb = stage.tile([P, F], bf16, tag="osb", bufs=4)
                    nc.vector.tensor_add(osb[:], pf[:],
                                         partials[nt * 2 + dh][:])
                    dq[(nt * 2 + dh) % 3].dma_start(
                        out=out_e[nt * P:(nt + 1) * P, ds], in_=osb[:])
    nc.compile()
    return nc


def _get_nc():
    if "nc" not in _CACHE:
        _CACHE["nc"] = _build()
    return _CACHE["nc"]


def _prep_shared(inputs):
    f = lambda a: np.asarray(a, np.float32)
    W = {}
    W["q"] = (f(inputs["Wq"]) + SCALING * (f(inputs["Bq"]) @ f(inputs["Aq"]))) * SCALE
    W["k"] = f(inputs["Wk"]) + SCALING * (f(inputs["Bk"]) @ f(inputs["Ak"]))
    W["v"] = f(inputs["Wv"]) + SCALING * (f(inputs["Bv"]) @ f(inputs["Av"]))
    W["o"] = f(inputs["Wo"]) + SCALING * (f(inputs["Bo"]) @ f(inputs["Ao"]))
    shared = {}
    for k, nm in (("q", "WqT"), ("k", "WkT"), ("v", "WvT"), ("o", "WoT")):
        shared[nm] = np.ascontiguousarray(W[k].T.astype(BF16))
    bo = f(inputs["bo"]).reshape(1, D)
    shared["boR"] = np.ascontiguousarray(
        np.broadcast_to(bo, (P, D)).astype(BF16))
    return shared


def kernel(**inputs):
    from concourse import bass_utils

    nc = _get_nc()
    shared = _prep_shared(inputs)
    x = np.asarray(inputs["x"], np.float32)
    in_maps = []
    for i in range(NCORES):
        m = dict(shared)
        m["xT"] = np.ascontiguousarray(x[i].T.astype(BF16))
        in_maps.append(m)
    res = bass_utils.run_bass_kernel_spmd(nc, in_maps,
                                          core_ids=list(range(NCORES)))
    return np.stack([np.asarray(res.results[i]["out"]).astype(np.float32)
                     for i in range(NCORES)], axis=0)


# revision 21
# speedup vs baseline: 1.1994x; 1.0237x over previous
"""LoRA multi-head attention on 8 TRN2 NeuronCores.

Sharding: data-parallel over batch (B=8 -> 1 batch element per core),
weights replicated, no collectives.

Host side: LoRA is folded into the dense weights exactly
(W' = W + (alpha/r) * B @ A), the attention scale 1/8 is folded into
Wq (power of two => lossless), and bo is pre-replicated across
partitions. The device then runs a plain dense MHA in bf16 with fp32
PSUM accumulation.

Device pipeline per core:
  qT/kT per dout-tile dt: (WT.T @ xT); v natural per token-tile with a
  ones column ([v_h | 1]) so PV also yields softmax denominators.
  Attention per dt (2 heads), per query-half nh, per key-tile mt:
    S^T pair via 64-row PE tiling (both heads concurrent) -> exp on
    ACT (psum -> bf16 sbuf) -> PV (M=65).  The next dout-tile's q/k
    projection matmuls are woven between S and PV so the PE never
    waits on the ACT exp latency.
  Normalization: reciprocal of the denom row (bf16) broadcast via a
  K=1 bf16 matmul; DVE multiply writes attnT directly (head 0) or via
  a staging tile + SBUF DMA (head 1).
  Output projection: kt=0..6 partial sums woven into the dt=7
  attention loop (+bo), tail adds only the kt=7 term.
"""

import sys

if "/opt/trn_rl_repo" not in sys.path:
    sys.path.insert(0, "/opt/trn_rl_repo")

import numpy as np
import ml_dtypes

BF16 = ml_dtypes.bfloat16

N = 1024  # tokens
D = 1024  # model dim
H = 16    # heads
HD = 64   # head dim
R = 16    # lora rank
P = 128   # partitions
F = 512   # psum free-dim tile
NCORES = 8
SCALING = 1.0 / 16.0  # lora alpha/rank
SCALE = HD ** -0.5

_CACHE = {}


def _build():
    import concourse.bacc as bacc
    import concourse.mybir as mybir
    import concourse.tile as tile

    f32 = mybir.dt.float32
    bf16 = mybir.dt.bfloat16
    Exp = mybir.ActivationFunctionType.Exp

    nc = bacc.Bacc("TRN2", target_bir_lowering=False, debug=False)

    xT_e = nc.declare_dram_parameter("xT", [D, N], bf16, isOutput=False)
    wT_e = {
        nm: nc.declare_dram_parameter(nm, [D, D], bf16, isOutput=False)
        for nm in ("WqT", "WkT", "WvT", "WoT")
    }
    bo_e = nc.declare_dram_parameter("boR", [P, D], bf16, isOutput=False)
    out_e = nc.declare_dram_parameter("out", [N, D], bf16, isOutput=True)

    with tile.TileContext(nc) as tc:
        with (
            tc.tile_pool(name="wpool", bufs=1) as wpool,
            tc.tile_pool(name="stage", bufs=2) as stage,
            tc.tile_pool(name="ps", bufs=1, space="PSUM") as ps,
        ):
            # ---- DMA loads: x first (5 queues), then weights ----
            dq = [nc.sync, nc.scalar, nc.gpsimd]
            qi = 0
            T = {}

            def load_big(nm, ext, qs3=False):
                nonlocal qi
                T[nm] = []
                for t in range(8):
                    tt = wpool.tile([P, D], bf16, tag=f"T_{nm}_{t}",
                                    name=f"T_{nm}_{t}")
                    eng = dq[qi % 3] if qs3 else [nc.sync, nc.gpsimd][qi % 2]
                    eng.dma_start(out=tt[:], in_=ext[t * P:(t + 1) * P, :])
                    qi += 1
                    T[nm].append(tt)

            # x spread over 3 queues (gates everything); Wq/Wk dt=0/1
            # columns sliced so dt=0 projections start at ~9us; Wv next
            # (v-projection); the rest as big region loads behind compute.
            # Steady-state queue roles: sync=ast/out, scalar=exp + dn0,
            # gpsimd=partition_broadcast.
            load_big("x", xT_e, qs3=True)
            wq2 = [nc.sync, nc.gpsimd]

            def load_qk_cols(dts):
                nonlocal qi
                c0, c1 = dts[0] * P, dts[-1] * P + P
                for nm in ("Wq", "Wk"):
                    if nm not in T:
                        T[nm] = [wpool.tile([P, D], bf16,
                                            tag=f"T_{nm}_{t}",
                                            name=f"T_{nm}_{t}")
                                 for t in range(8)]
                    for t in range(8):
                        wq2[qi % 2].dma_start(
                            out=T[nm][t][:, c0:c1],
                            in_=wT_e[nm + "T"][t * P:(t + 1) * P, c0:c1])
                        qi += 1

            load_qk_cols([0])
            load_qk_cols([1])
            load_big("Wv", wT_e["WvT"])
            load_qk_cols([2, 3, 4, 5, 6, 7])
            load_big("Wo", wT_e["WoT"])
            bo_sb = wpool.tile([P, D], bf16, tag="bo")
            nc.gpsimd.dma_start(out=bo_sb[:], in_=bo_e[:, :])

            ones1 = wpool.tile([1, HD], bf16, tag="ones1")
            nc.vector.memset(ones1[:], 1.0)
            warm = wpool.tile([P, F], bf16, tag="warm")
            nc.vector.memset(warm[:], 0.0)

            # ---- ACT exp-table preload + PE warm-up (no DMA deps) ----
            wexp = stage.tile([P, F], bf16, tag="wexp", bufs=1)
            wps = ps.tile([P, F], f32, tag="wk", bufs=2)
            nc.tensor.matmul(wps[:], warm[:, 0:P], warm[:], start=True,
                             stop=True)
            nc.scalar.activation(wexp[:], wps[:], Exp)
            for _ in range(23):
                nc.tensor.matmul(wps[:], warm[:, 0:P], warm[:],
                                 start=True, stop=True)

            # ---- q/k projection generator for dout tile dt ----
            qks = {}

            def proj_gen(dt):
                qk = {}
                for nm, wnm in (("q", "Wq"), ("k", "Wk")):
                    dst = wpool.tile([P, D], bf16, tag=f"{nm}T",
                                     bufs=3, name=f"{nm}T_{dt}")
                    qk[nm] = dst
                    for nh in range(2):
                        ns = slice(nh * F, (nh + 1) * F)
                        pq = ps.tile([P, F], f32, tag="wk", bufs=2)
                        for kt in range(8):
                            nc.tensor.matmul(
                                pq[:], T[wnm][kt][:, dt * P:(dt + 1) * P],
                                T["x"][kt][:, ns],
                                start=(kt == 0), stop=(kt == 7))
                            yield
                        nc.vector.tensor_copy(dst[:, ns], pq[:])
                        yield
                qks[dt] = qk

            # ---- output-projection partials (kt=0..6) for dt=7 weave ----
            attnT = [wpool.tile([P, D], bf16, tag=f"attnT_{t}",
                                name=f"attnT_{t}") for t in range(8)]
            partials = [wpool.tile([P, F], bf16, tag=f"part_{t}",
                                   name=f"part_{t}") for t in range(16)]

            def out_gen():
                for nt in range(8):
                    for dh in range(2):
                        ds = slice(dh * F, (dh + 1) * F)
                        pf = ps.tile([P, F], f32, tag="wk", bufs=2)
                        for kt in range(7):
                            nc.tensor.matmul(
                                pf[:], attnT[kt][:, nt * P:(nt + 1) * P],
                                T["Wo"][kt][:, ds],
                                start=(kt == 0), stop=(kt == 6))
                            yield
                        nc.vector.tensor_add(
                            partials[nt * 2 + dh][:], pf[:], bo_sb[:, ds])
                        yield

            # ---- S-pair + exp issue (attention front half) ----
            PTE_BUFS = 13
            pmap = {}

            def s_exp(dt, nh, mt):
                qt = qks[dt]["q"]
                ktt = qks[dt]["k"]
                ns = slice(nh * F, (nh + 1) * F)
                m0 = mt * P
                spair = ps.tile([P, 2 * F], f32, tag="spair", bufs=2)
                nc.tensor.matmul(spair[:, 0:F], ktt[0:HD, m0:m0 + P],
                                 qt[0:HD, ns], start=True, stop=True)
                nc.tensor.matmul(spair[:, F:2 * F], ktt[HD:P, m0:m0 + P],
                                 qt[HD:P, ns], start=True, stop=True)
                pte = stage.tile([P, 2 * F], bf16, tag="pte",
                                 bufs=PTE_BUFS)
                nc.scalar.activation(pte[:], spair[:], Exp)
                pmap[(dt, nh, mt)] = pte

            # ---- v projection with dt=0 S/exp pre-issue woven in ----
            VW = H * (HD + 1)  # 1040
            v_sb = [wpool.tile([P, VW], bf16, tag=f"v_{t}",
                               name=f"v_{t}") for t in range(8)]
            g0 = proj_gen(0)
            for _ in g0:
                pass

            sched0 = [(0, nh, mt) for nh in range(2) for mt in range(8)]
            s0 = 0
            for nt in range(8):
                vr = v_sb[nt][:].rearrange("p (h c) -> p h c", c=HD + 1)
                for dh in range(2):
                    ds = slice(dh * F, (dh + 1) * F)
                    pv = ps.tile([P, F], f32, tag="wk", bufs=2)
                    for kt in range(8):
                        nc.tensor.matmul(
                            pv[:], T["x"][kt][:, nt * P:(nt + 1) * P],
                            T["Wv"][kt][:, ds], start=(kt == 0),
                            stop=(kt == 7))
                    pvr = pv[:].rearrange("p (h c) -> p h c", c=HD)
                    nc.vector.tensor_copy(vr[:, dh * 8:(dh + 1) * 8, 0:HD],
                                          pvr[:])
                    if s0 < 12 and nt >= 1:
                        s_exp(*sched0[s0])
                        s0 += 1
                nc.vector.memset(vr[:, :, HD:HD + 1], 1.0)

            # ---- attention + weave ----
            def norm(dt, nh, po0, po1):
                ns = slice(nh * F, (nh + 1) * F)
                for hi, po in ((0, po0), (1, po1)):
                    # one copy frees the po PSUM bank; rest runs from SBUF
                    oah = stage.tile([HD + 1, F], f32, tag="oah", bufs=4)
                    nc.vector.tensor_copy(oah[:], po[:])
                    dn0 = stage.tile([1, F], f32, tag="dn0", bufs=3)
                    nc.scalar.dma_start(out=dn0[:], in_=oah[HD:HD + 1, :])
                    rec32 = stage.tile([1, F], f32, tag="rec32", bufs=3)
                    nc.vector.reciprocal_approx_fast(rec32[:], dn0[:])
                    pbs = stage.tile([HD, F], f32, tag="pbs", bufs=3)
                    nc.gpsimd.partition_broadcast(pbs[:], rec32[0:1, :])
                    if hi == 0:
                        nc.vector.tensor_mul(attnT[dt][0:HD, ns],
                                             oah[0:HD, :], pbs[:])
                    else:
                        ast = stage.tile([HD, F], bf16, tag="ast", bufs=3)
                        nc.vector.tensor_mul(ast[:], oah[0:HD, :],
                                             pbs[:])
                        nc.scalar.dma_start(out=attnT[dt][HD:P, ns],
                                            in_=ast[:])

            # Flat software pipeline over all 128 (dt, nh, mt) steps:
            # S(j) issues one step ahead of PV(j-1) so ACT runs exp
            # back-to-back across block boundaries.
            steps = [(dt, nh, mt) for dt in range(8) for nh in range(2)
                     for mt in range(8)]
            gens = {dt: (proj_gen(dt + 1) if dt < 7 else out_gen())
                    for dt in range(8)}
            pos = {}
            for j in range(129):
                if j >= 1 and j < 128 and steps[j - 1][1:] == (1, 7):
                    # dt boundary: finish the next dt's q/k projection
                    # (python-level: binds qks[dt+1]) before issuing its S
                    for _ in gens[steps[j - 1][0]]:
                        pass
                if j < 128:
                    dt, nh, mt = steps[j]
                    if mt == 0:
                        pos[(dt, nh)] = (
                            ps.tile([HD + 1, F], f32, tag="po", bufs=2,
                                    name=f"po0_{dt}_{nh}"),
                            ps.tile([HD + 1, F], f32, tag="po", bufs=2,
                                    name=f"po1_{dt}_{nh}"))
                    if not (dt == 0 and (nh * 8 + mt) < 12):
                        s_exp(dt, nh, mt)
                if j >= 1:
                    dt, nh, mt = steps[j - 1]
                    g = gens[dt]
                    nw = 8 if dt == 7 else 2
                    for _ in range(nw):
                        next(g, None)
                    h0 = 2 * dt
                    po0, po1 = pos[(dt, nh)]
                    pte = pmap.pop((dt, nh, mt))
                    nc.tensor.matmul(
                        po0[:], v_sb[mt][:, h0 * (HD + 1):
                                         (h0 + 1) * (HD + 1)],
                        pte[:, 0:F], start=(mt == 0), stop=(mt == 7))
                    nc.tensor.matmul(
                        po1[:], v_sb[mt][:, (h0 + 1) * (HD + 1):
                                         (h0 + 2) * (HD + 1)],
                        pte[:, F:2 * F], start=(mt == 0), stop=(mt == 7))
                    if mt == 7:
                        norm(dt, nh, po0, po1)

            # ---- output projection tail: kt=7 term + partial ----
            for nt in range(8):
                for dh in range(2):
                    ds = slice(dh * F, (dh + 1) * F)
                    pf = ps.tile([P, F], f32, tag="wk", bufs=2)
                    nc.tensor.matmul(pf[:],
                                     attnT[7][:, nt * P:(nt + 1) * P],
                                     T["Wo"][7][:, ds],
                                     start=True, stop=True)
                    osb = stage.tile([P, F], bf16, tag="osb", bufs=4)
                    nc.vector.tensor_add(osb[:], pf[:],
                                         partials[nt * 2 + dh][:])
                    dq[(nt * 2 + dh) % 3].dma_start(
                        out=out_e[nt * P:(nt + 1) * P, ds], in_=osb[:])
    nc.compile()
    return nc


def _get_nc():
    if "nc" not in _CACHE:
        _CACHE["nc"] = _build()
    return _CACHE["nc"]


def _prep_shared(inputs):
    f = lambda a: np.asarray(a, np.float32)
    W = {}
    W["q"] = (f(inputs["Wq"]) + SCALING * (f(inputs["Bq"]) @ f(inputs["Aq"]))) * SCALE
    W["k"] = f(inputs["Wk"]) + SCALING * (f(inputs["Bk"]) @ f(inputs["Ak"]))
    W["v"] = f(inputs["Wv"]) + SCALING * (f(inputs["Bv"]) @ f(inputs["Av"]))
    W["o"] = f(inputs["Wo"]) + SCALING * (f(inputs["Bo"]) @ f(inputs["Ao"]))
    shared = {}
    for k, nm in (("q", "WqT"), ("k", "WkT"), ("v", "WvT"), ("o", "WoT")):
        shared[nm] = np.ascontiguousarray(W[k].T.astype(BF16))
    bo = f(inputs["bo"]).reshape(1, D)
    shared["boR"] = np.ascontiguousarray(
        np.broadcast_to(bo, (P, D)).astype(BF16))
    return shared


def kernel(**inputs):
    from concourse import bass_utils

    nc = _get_nc()
    shared = _prep_shared(inputs)
    x = np.asarray(inputs["x"], np.float32)
    in_maps = []
    for i in range(NCORES):
        m = dict(shared)
        m["xT"] = np.ascontiguousarray(x[i].T.astype(BF16))
        in_maps.append(m)
    res = bass_utils.run_bass_kernel_spmd(nc, in_maps,
                                          core_ids=list(range(NCORES)))
    return np.stack([np.asarray(res.results[i]["out"]).astype(np.float32)
                     for i in range(NCORES)], axis=0)


# revision 25
# speedup vs baseline: 1.2502x; 1.0424x over previous
"""LoRA multi-head attention on 8 TRN2 NeuronCores.

Sharding: data-parallel over batch (B=8 -> 1 batch element per core),
weights replicated, no collectives.

Host side: LoRA is folded into the dense weights exactly
(W' = W + (alpha/r) * B @ A), the attention scale 1/8 is folded into
Wq (power of two => lossless), and bo is pre-replicated across
partitions. The device then runs a plain dense MHA in bf16 with fp32
PSUM accumulation.

Device pipeline per core:
  qT/kT per dout-tile dt: (WT.T @ xT); v natural per token-tile with a
  ones column ([v_h | 1]) so PV also yields softmax denominators.
  Attention per dt (2 heads), per query-half nh, per key-tile mt:
    S^T pair via 64-row PE tiling (both heads concurrent) -> exp on
    ACT (psum -> bf16 sbuf) -> PV (M=65).  The next dout-tile's q/k
    projection matmuls are woven between S and PV so the PE never
    waits on the ACT exp latency.
  Normalization: reciprocal of the denom row (bf16) broadcast via a
  K=1 bf16 matmul; DVE multiply writes attnT directly (head 0) or via
  a staging tile + SBUF DMA (head 1).
  Output projection: kt=0..6 partial sums woven into the dt=7
  attention loop (+bo), tail adds only the kt=7 term.
"""

import sys

if "/opt/trn_rl_repo" not in sys.path:
    sys.path.insert(0, "/opt/trn_rl_repo")

import numpy as np
import ml_dtypes

BF16 = ml_dtypes.bfloat16

N = 1024  # tokens
D = 1024  # model dim
H = 16    # heads
HD = 64   # head dim
R = 16    # lora rank
P = 128   # partitions
F = 512   # psum free-dim tile
NCORES = 8
SCALING = 1.0 / 16.0  # lora alpha/rank
SCALE = HD ** -0.5

_CACHE = {}


def _build():
    import concourse.bacc as bacc
    import concourse.mybir as mybir
    import concourse.tile as tile

    f32 = mybir.dt.float32
    bf16 = mybir.dt.bfloat16
    Exp = mybir.ActivationFunctionType.Exp

    nc = bacc.Bacc("TRN2", target_bir_lowering=False, debug=False)

    xT_e = nc.declare_dram_parameter("xT", [D, N], bf16, isOutput=False)
    wT_e = {
        nm: nc.declare_dram_parameter(nm, [D, D], bf16, isOutput=False)
        for nm in ("WqT", "WkT", "WvT", "WoT")
    }
    bo_e = nc.declare_dram_parameter("boR", [P, D], bf16, isOutput=False)
    out_e = nc.declare_dram_parameter("out", [N, D], bf16, isOutput=True)

    with tile.TileContext(nc) as tc:
        with (
            tc.tile_pool(name="wpool", bufs=1) as wpool,
            tc.tile_pool(name="stage", bufs=2) as stage,
            tc.tile_pool(name="ps", bufs=1, space="PSUM") as ps,
        ):
            # ---- DMA loads: x first (5 queues), then weights ----
            dq = [nc.sync, nc.scalar, nc.gpsimd]
            qi = 0
            T = {}

            def load_big(nm, ext, qs3=False):
                nonlocal qi
                T[nm] = []
                for t in range(8):
                    tt = wpool.tile([P, D], bf16, tag=f"T_{nm}_{t}",
                                    name=f"T_{nm}_{t}")
                    eng = dq[qi % 3] if qs3 else [nc.sync, nc.gpsimd][qi % 2]
                    eng.dma_start(out=tt[:], in_=ext[t * P:(t + 1) * P, :])
                    qi += 1
                    T[nm].append(tt)

            # x spread over 3 queues (gates everything); Wq/Wk dt=0/1
            # columns sliced so dt=0 projections start at ~9us; Wv next
            # (v-projection); the rest as big region loads behind compute.
            # Steady-state queue roles: sync=ast/out, scalar=exp + dn0,
            # gpsimd=partition_broadcast.
            load_big("x", xT_e, qs3=True)
            wq2 = [nc.sync, nc.gpsimd]

            def load_qk_cols(dts):
                nonlocal qi
                c0, c1 = dts[0] * P, dts[-1] * P + P
                for nm in ("Wq", "Wk"):
                    if nm not in T:
                        T[nm] = [wpool.tile([P, D], bf16,
                                            tag=f"T_{nm}_{t}",
                                            name=f"T_{nm}_{t}")
                                 for t in range(8)]
                    for t in range(8):
                        wq2[qi % 2].dma_start(
                            out=T[nm][t][:, c0:c1],
                            in_=wT_e[nm + "T"][t * P:(t + 1) * P, c0:c1])
                        qi += 1

            load_qk_cols([0])
            load_big("Wv", wT_e["WvT"])
            load_qk_cols([1])
            load_qk_cols([2, 3, 4, 5, 6, 7])
            load_big("Wo", wT_e["WoT"])
            bo_sb = wpool.tile([P, D], bf16, tag="bo")
            nc.gpsimd.dma_start(out=bo_sb[:], in_=bo_e[:, :])

            ones1 = wpool.tile([1, HD], bf16, tag="ones1")
            nc.vector.memset(ones1[:], 1.0)
            warm = wpool.tile([P, F], bf16, tag="warm")
            nc.vector.memset(warm[:], 0.0)

            # ---- ACT exp-table preload + PE warm-up (no DMA deps) ----
            wexp = stage.tile([P, F], bf16, tag="wexp", bufs=1)
            wps = ps.tile([P, F], f32, tag="wk", bufs=2)
            nc.tensor.matmul(wps[:], warm[:, 0:P], warm[:], start=True,
                             stop=True)
            nc.scalar.activation(wexp[:], wps[:], Exp)
            for _ in range(23):
                nc.tensor.matmul(wps[:], warm[:, 0:P], warm[:],
                                 start=True, stop=True)

            # ---- q/k projection generator for dout tile dt ----
            qks = {}

            def proj_gen(dt):
                qk = {}
                for nm, wnm in (("q", "Wq"), ("k", "Wk")):
                    dst = wpool.tile([P, D], bf16, tag=f"{nm}T",
                                     bufs=3, name=f"{nm}T_{dt}")
                    qk[nm] = dst
                    for nh in range(2):
                        ns = slice(nh * F, (nh + 1) * F)
                        pq = ps.tile([P, F], f32, tag="wk", bufs=2)
                        for kt in range(8):
                            nc.tensor.matmul(
                                pq[:], T[wnm][kt][:, dt * P:(dt + 1) * P],
                                T["x"][kt][:, ns],
                                start=(kt == 0), stop=(kt == 7))
                            yield
                        nc.vector.tensor_copy(dst[:, ns], pq[:])
                        yield
                qks[dt] = qk

            # ---- output-projection partials (kt=0..6) for dt=7 weave ----
            attnT = [wpool.tile([P, D], bf16, tag=f"attnT_{t}",
                                name=f"attnT_{t}") for t in range(8)]
            partials = [wpool.tile([P, F], bf16, tag=f"part_{t}",
                                   name=f"part_{t}") for t in range(16)]

            def out_gen():
                for nt in range(8):
                    for dh in range(2):
                        ds = slice(dh * F, (dh + 1) * F)
                        pf = ps.tile([P, F], f32, tag="wk", bufs=2)
                        for kt in range(7):
                            nc.tensor.matmul(
                                pf[:], attnT[kt][:, nt * P:(nt + 1) * P],
                                T["Wo"][kt][:, ds],
                                start=(kt == 0), stop=(kt == 6))
                            yield
                        nc.vector.tensor_add(
                            partials[nt * 2 + dh][:], pf[:], bo_sb[:, ds])
                        yield

            # ---- S-pair + exp issue (attention front half) ----
            PTE_BUFS = 15
            pmap = {}

            def s_exp(dt, nh, mt):
                qt = qks[dt]["q"]
                ktt = qks[dt]["k"]
                ns = slice(nh * F, (nh + 1) * F)
                m0 = mt * P
                spair = ps.tile([P, 2 * F], f32, tag="spair", bufs=2)
                nc.tensor.matmul(spair[:, 0:F], ktt[0:HD, m0:m0 + P],
                                 qt[0:HD, ns], start=True, stop=True)
                nc.tensor.matmul(spair[:, F:2 * F], ktt[HD:P, m0:m0 + P],
                                 qt[HD:P, ns], start=True, stop=True)
                pte = stage.tile([P, 2 * F], bf16, tag="pte",
                                 bufs=PTE_BUFS)
                nc.scalar.activation(pte[:], spair[:], Exp)
                pmap[(dt, nh, mt)] = pte

            # ---- v projection with dt=0 S/exp pre-issue woven in ----
            VW = H * (HD + 1)  # 1040
            v_sb = [wpool.tile([P, VW], bf16, tag=f"v_{t}",
                               name=f"v_{t}") for t in range(8)]
            g0 = proj_gen(0)
            for _ in g0:
                pass

            sched0 = [(0, nh, mt) for nh in range(2) for mt in range(8)]
            s0 = 0
            for nt in range(8):
                vr = v_sb[nt][:].rearrange("p (h c) -> p h c", c=HD + 1)
                for dh in range(2):
                    ds = slice(dh * F, (dh + 1) * F)
                    pv = ps.tile([P, F], f32, tag="wk", bufs=2)
                    for kt in range(8):
                        nc.tensor.matmul(
                            pv[:], T["x"][kt][:, nt * P:(nt + 1) * P],
                            T["Wv"][kt][:, ds], start=(kt == 0),
                            stop=(kt == 7))
                    pvr = pv[:].rearrange("p (h c) -> p h c", c=HD)
                    nc.vector.tensor_copy(vr[:, dh * 8:(dh + 1) * 8, 0:HD],
                                          pvr[:])
                    if s0 < 12 and nt >= 1:
                        s_exp(*sched0[s0])
                        s0 += 1
                nc.vector.memset(vr[:, :, HD:HD + 1], 1.0)

            # ---- attention + weave ----
            def norm(dt, nh, po0, po1):
                ns = slice(nh * F, (nh + 1) * F)
                for hi, po in ((0, po0), (1, po1)):
                    # one copy frees the po PSUM bank; rest runs from SBUF
                    oah = stage.tile([HD + 1, F], f32, tag="oah", bufs=4)
                    nc.vector.tensor_copy(oah[:], po[:])
                    dn0 = stage.tile([1, F], f32, tag="dn0", bufs=3)
                    nc.scalar.dma_start(out=dn0[:], in_=oah[HD:HD + 1, :])
                    rec32 = stage.tile([1, F], f32, tag="rec32", bufs=3)
                    nc.vector.reciprocal_approx_fast(rec32[:], dn0[:])
                    pbs = stage.tile([HD, F], f32, tag="pbs", bufs=3)
                    nc.gpsimd.partition_broadcast(pbs[:], rec32[0:1, :])
                    if hi == 0:
                        nc.vector.tensor_mul(attnT[dt][0:HD, ns],
                                             oah[0:HD, :], pbs[:])
                    else:
                        ast = stage.tile([HD, F], bf16, tag="ast", bufs=3)
                        nc.vector.tensor_mul(ast[:], oah[0:HD, :],
                                             pbs[:])
                        nc.scalar.dma_start(out=attnT[dt][HD:P, ns],
                                            in_=ast[:])

            # Flat software pipeline over all 128 (dt, nh, mt) steps:
            # S(j) issues one step ahead of PV(j-1) so ACT runs exp
            # back-to-back across block boundaries.
            steps = [(dt, nh, mt) for dt in range(8) for nh in range(2)
                     for mt in range(8)]
            gens = {dt: (proj_gen(dt + 1) if dt < 7 else out_gen())
                    for dt in range(8)}
            # PV lags S by 2 steps so the exp latency (plus semaphore
            # jitter) never stalls the PE queue.
            LAG = 2
            pos = {}
            for j in range(128 + LAG):
                if 1 <= j < 128 and steps[j][1:] == (0, 0) and steps[j][0]:
                    # dt boundary: finish the next dt's q/k projection
                    # (python-level: binds qks[dt]) before issuing its S
                    for _ in gens[steps[j][0] - 1]:
                        pass
                if j < 128:
                    dt, nh, mt = steps[j]
                    if mt == 0:
                        pos[(dt, nh)] = (
                            ps.tile([HD + 1, F], f32, tag="po", bufs=2,
                                    name=f"po0_{dt}_{nh}"),
                            ps.tile([HD + 1, F], f32, tag="po", bufs=2,
                                    name=f"po1_{dt}_{nh}"))
                    if not (dt == 0 and (nh * 8 + mt) < 12):
                        s_exp(dt, nh, mt)
                if j >= LAG:
                    dt, nh, mt = steps[j - LAG]
                    g = gens[dt]
                    nw = 8 if dt == 7 else 2
                    for _ in range(nw):
                        next(g, None)
                    h0 = 2 * dt
                    po0, po1 = pos[(dt, nh)]
                    pte = pmap.pop((dt, nh, mt))
                    nc.tensor.matmul(
                        po0[:], v_sb[mt][:, h0 * (HD + 1):
                                         (h0 + 1) * (HD + 1)],
                        pte[:, 0:F], start=(mt == 0), stop=(mt == 7))
                    nc.tensor.matmul(
                        po1[:], v_sb[mt][:, (h0 + 1) * (HD + 1):
                                         (h0 + 2) * (HD + 1)],
                        pte[:, F:2 * F], start=(mt == 0), stop=(mt == 7))
                    if mt == 7:
                        norm(dt, nh, po0, po1)

            # ---- output projection tail: kt=7 term + partial ----
            for nt in range(8):
                for dh in range(2):
                    ds = slice(dh * F, (dh + 1) * F)
                    pf = ps.tile([P, F], f32, tag="wk", bufs=2)
                    nc.tensor.matmul(pf[:],
                                     attnT[7][:, nt * P:(nt + 1) * P],
                                     T["Wo"][7][:, ds],
                                     start=True, stop=True)
                    osb = stage.tile([P, F], bf16, tag="osb", bufs=4)
                    nc.vector.tensor_add(osb[:], pf[:],
                                         partials[nt * 2 + dh][:])
                    dq[(nt * 2 + dh) % 3].dma_start(
                        out=out_e[nt * P:(nt + 1) * P, ds], in_=osb[:])
    nc.compile()
    return nc


def _get_nc():
    if "nc" not in _CACHE:
        _CACHE["nc"] = _build()
    return _CACHE["nc"]


def _prep_shared(inputs):
    f = lambda a: np.asarray(a, np.float32)
    W = {}
    W["q"] = (f(inputs["Wq"]) + SCALING * (f(inputs["Bq"]) @ f(inputs["Aq"]))) * SCALE
    W["k"] = f(inputs["Wk"]) + SCALING * (f(inputs["Bk"]) @ f(inputs["Ak"]))
    W["v"] = f(inputs["Wv"]) + SCALING * (f(inputs["Bv"]) @ f(inputs["Av"]))
    W["o"] = f(inputs["Wo"]) + SCALING * (f(inputs["Bo"]) @ f(inputs["Ao"]))
    shared = {}
    for k, nm in (("q", "WqT"), ("k", "WkT"), ("v", "WvT"), ("o", "WoT")):
        shared[nm] = np.ascontiguousarray(W[k].T.astype(BF16))
    bo = f(inputs["bo"]).reshape(1, D)
    shared["boR"] = np.ascontiguousarray(
        np.broadcast_to(bo, (P, D)).astype(BF16))
    return shared


def kernel(**inputs):
    from concourse import bass_utils

    nc = _get_nc()
    shared = _prep_shared(inputs)
    x = np.asarray(inputs["x"], np.float32)
    in_maps = []
    for i in range(NCORES):
        m = dict(shared)
        m["xT"] = np.ascontiguousarray(x[i].T.astype(BF16))
        in_maps.append(m)
    res = bass_utils.run_bass_kernel_spmd(nc, in_maps,
                                          core_ids=list(range(NCORES)))
    return np.stack([np.asarray(res.results[i]["out"]).astype(np.float32)
                     for i in range(NCORES)], axis=0)
